# revision 6
# baseline (speedup 1.0000x reference)
"""CrissCrossAttention Trainium2 kernel.

Math notes (verified in float64): the reference's column-attention einsum
('bnjid,bnkid->bnjik' applied to grid-swapped q/k/v) is an alpha-renaming
that exactly undoes the swap, so reference == 2 * row_attention:
    out = (2 * row_attn(x)) @ Wo + bo
Row attention per (batch, head, grid-row i):
    S = Q_i K_i^T * d^-0.5 ; P = softmax_k(S) ; O_i = P V_i
with grid 64x64 (n = 4096 = i*64 + j), heads=8, d=64.

Distribution: data-parallel over batch; core b handles x[b].

Per-core pipeline (all in one Bass/Tile program):
  x -> xT (PE transposes)                      [fp32]
  qT/kT = W^T x^T (fp32r matmuls, d-major)     -> bf16
  v = x W_v (fp32r matmuls, n-major)           -> bf16
  S[j,k] per (head, i) via 64x64x64 matmuls packed 2-heads/psum-bank
  P = exp(S/8) * (1/rowsum)  (ACT exp + DVE reduce/recip/mul)
  P^T via PE pair-transposes ; O^T = V^T-style matmul with cross-head garbage
  OT assembled d-major -> final out = OT^T (2*Wo) + bo (bf16 matmul)
"""

import sys

if "/opt/trn_rl_repo" not in sys.path:
    sys.path.insert(0, "/opt/trn_rl_repo")

import numpy as np

import concourse.bass as bass
import concourse.mybir as mybir
import concourse.tile as tile
from concourse import bacc
from concourse.masks import make_identity

F32 = mybir.dt.float32
F32R = mybir.dt.float32r
BF16 = mybir.dt.bfloat16

N = 4096
D = 512
G = 64          # grid side
NH = 8          # heads
DH = 64         # head dim
NC = 4          # dim chunks of 128 (2 heads each)
NT = 32         # n tiles of 128
NB = 8          # n banks of 512
SCALE = DH ** -0.5


def r(ap):
    """Bitcast a float32 AP to float32r for full-rate PE matmuls."""
    return ap.bitcast(F32R)


def build_kernel(n_cores: int = 8):
    nc = bacc.Bacc("TRN2", target_bir_lowering=False, debug=False,
                   num_devices=n_cores)

    x_d = nc.dram_tensor("x", [N, D], F32, kind="ExternalInput").ap()
    wq_d = nc.dram_tensor("Wq", [D, D], F32, kind="ExternalInput").ap()
    wk_d = nc.dram_tensor("Wk", [D, D], F32, kind="ExternalInput").ap()
    wv_d = nc.dram_tensor("Wv", [D, D], F32, kind="ExternalInput").ap()
    wo_d = nc.dram_tensor("Wo", [D, D], F32, kind="ExternalInput").ap()
    bo_d = nc.dram_tensor("bo", [D], F32, kind="ExternalInput").ap()
    out_d = nc.dram_tensor("out", [N, D], F32, kind="ExternalOutput").ap()

    with tile.TileContext(nc) as tc:
        with (
            tc.tile_pool(name="consts", bufs=1) as consts,
            tc.tile_pool(name="psum", bufs=8, space="PSUM") as psum,
            tc.tile_pool(name="qk", bufs=1) as qkpool,
            tc.tile_pool(name="vpool", bufs=1) as vpool,
            tc.tile_pool(name="attn", bufs=3) as attn,
            tc.tile_pool(name="outsb", bufs=3) as outp,
        ):
            ident_f32 = consts.tile([128, 128], F32, tag="idf")
            make_identity(nc, ident_f32)
            ident_bf = consts.tile([128, 128], BF16, tag="idb")
            make_identity(nc, ident_bf)

            qT = [qkpool.tile([128, N], BF16, tag=f"qT{c}", name=f"qT{c}") for c in range(NC)]
            kT = [qkpool.tile([128, N], BF16, tag=f"kT{c}", name=f"kT{c}") for c in range(NC)]
            v_sb = vpool.tile([128, NT, D], BF16, tag="v")

            # ---- phase 0+1: x load, transpose, projections -------------
            with (
                tc.tile_pool(name="xt", bufs=1) as xtpool,
                tc.tile_pool(name="xin", bufs=4) as xin,
                tc.tile_pool(name="wpool", bufs=1) as wpool,
            ):
                xT = [xtpool.tile([128, N], F32R, tag=f"xT{k}", name=f"xT{k}")
                      for k in range(NC)]
                for nt in range(NT):
                    xtile = xin.tile([128, D], F32, tag="xtile")
                    nc.sync.dma_start(out=xtile, in_=x_d[nt * 128:(nt + 1) * 128, :])
                    tp = psum.tile([128, NC, 128], F32, tag="bank")
                    for kc in range(NC):
                        nc.tensor.transpose(
                            tp[:, kc, :], xtile[:, kc * 128:(kc + 1) * 128],
                            ident_f32)
                    for kc in range(NC):
                        nc.vector.tensor_copy(
                            out=xT[kc][:, nt * 128:(nt + 1) * 128],
                            in_=tp[:, kc, :])

                # weights: Wq and Wk resident together; Wv reuses Wq's slot
                wq_sb = wpool.tile([128, NC, D], F32R, tag="w0")
                nc.gpsimd.dma_start(
                    out=wq_sb, in_=wq_d.rearrange("(kc p) e -> p kc e", p=128))
                wk_sb = wpool.tile([128, NC, D], F32R, tag="w1")
                nc.gpsimd.dma_start(
                    out=wk_sb, in_=wk_d.rearrange("(kc p) e -> p kc e", p=128))

                # qT/kT: [dout-chunk 128, n] = W.T @ xT  (fp32r, N=512)
                for wsb, dst in ((wq_sb, qT), (wk_sb, kT)):
                    for c in range(NC):
                        for nb in range(NB):
                            pj = psum.tile([128, D], F32, tag="bank")
                            for kc in range(NC):
                                nc.tensor.matmul(
                                    pj,
                                    wsb[:, kc, c * 128:(c + 1) * 128],
                                    xT[kc][:, nb * D:(nb + 1) * D],
                                    start=(kc == 0), stop=(kc == NC - 1))
                            nc.scalar.copy(
                                out=dst[c][:, nb * D:(nb + 1) * D], in_=pj)

                wv_sb = wpool.tile([128, NC, D], F32R, tag="w0")
                nc.gpsimd.dma_start(
                    out=wv_sb, in_=wv_d.rearrange("(kc p) e -> p kc e", p=128))
                # v: [n-tile 128, 512] = xT-block.T @ Wv  (fp32r, N=512)
                for nt in range(NT):
                    pv = psum.tile([128, D], F32, tag="bank")
                    for kc in range(NC):
                        nc.tensor.matmul(
                            pv,
                            xT[kc][:, nt * 128:(nt + 1) * 128],
                            wv_sb[:, kc, :],
                            start=(kc == 0), stop=(kc == NC - 1))
                    nc.vector.tensor_copy(out=v_sb[:, nt, :], in_=pv)

            # ---- phase 2: row attention --------------------------------
            otpool_cm = tc.tile_pool(name="otpool", bufs=1)
            otpool = otpool_cm.__enter__()
            ot = [otpool.tile([128, N], BF16, tag=f"ot{c}", name=f"ot{c}")
                  for c in range(NC)]
            for c in range(NC):
                for bk in range(NB):
                    sb = psum.tile([128, 8, G], F32, tag="bank")
                    for s in range(8):
                        i = bk * 8 + s
                        for h in range(2):
                            p0 = h * 64
                            nc.tensor.matmul(
                                sb[p0:p0 + 64, s, :],
                                qT[c][p0:p0 + 64, i * G:(i + 1) * G],
                                kT[c][p0:p0 + 64, i * G:(i + 1) * G],
                                start=True, stop=True,
                                tile_position=(p0, p0))
                    eb = attn.tile([128, 8, G], BF16, tag="eb")
                    nc.scalar.activation(out=eb, in_=sb,
                                         func=mybir.ActivationFunctionType.Exp,
                                         scale=SCALE)
                    sums = attn.tile([128, 8], F32, tag="sums")
                    nc.vector.reduce_sum(out=sums, in_=eb,
                                         axis=mybir.AxisListType.X)
                    rec = attn.tile([128, 8], F32, tag="rec")
                    nc.vector.reciprocal(out=rec, in_=sums)
                    rec_b = bass.AP(tensor=rec.tensor, offset=rec.offset,
                                    ap=[rec.ap[0], rec.ap[1], [0, G]])
                    pb = attn.tile([128, 8, G], BF16, tag="pb")
                    nc.vector.tensor_mul(pb, eb, rec_b)

                    ptp = psum.tile([128, 4, 128], BF16, tag="bank")
                    for p2 in range(4):
                        nc.tensor.transpose(
                            ptp[:, p2, :], pb[:, 2 * p2:2 * p2 + 2, :],
                            ident_bf)
                    pts = attn.tile([128, 4, 128], BF16, tag="pts")
                    nc.vector.tensor_copy(out=pts, in_=ptp)

                    ob = [psum.tile([128, 4, 128], F32, tag="bank", name=f"ob{e}")
                          for e in range(2)]
                    for p2 in range(4):
                        for e in range(2):
                            i = bk * 8 + 2 * p2 + e
                            nc.tensor.matmul(
                                ob[e][:, p2, :],
                                v_sb[e * 64:e * 64 + 64, i // 2,
                                     c * 128:(c + 1) * 128],
                                pts[e * 64:e * 64 + 64, p2, :],
                                start=True, stop=True,
                                tile_position=(e * 64, 0))
                    # extract diagonal (per-head) blocks into OT
                    pstride = ot[c].ap[0][0]
                    for e in range(2):
                        for h in range(2):
                            src = ob[e][h * 64:h * 64 + 64, :, h * 64:h * 64 + 64]
                            dst = bass.AP(
                                tensor=ot[c].tensor,
                                offset=(ot[c].offset + h * 64 * pstride
                                        + (bk * 8 + e) * G),
                                ap=[[pstride, 64], [2 * G, 4], [1, G]])
                            nc.vector.tensor_copy(out=dst, in_=src)

            # ---- phase 3: output projection ----------------------------
            with tc.tile_pool(name="wout", bufs=1) as wout:
                wo_f32 = wout.tile([128, NC, D], F32, tag="wof")
                nc.gpsimd.dma_start(
                    out=wo_f32, in_=wo_d.rearrange("(kc p) e -> p kc e", p=128))
                wo_bf = wout.tile([128, NC, D], BF16, tag="wob")
                # fold the criss-cross 2x into Wo
                nc.scalar.mul(out=wo_bf, in_=wo_f32, mul=2.0)
                bo128 = wout.tile([128, D], F32, tag="bo")
                nc.sync.dma_start(
                    out=bo128,
                    in_=bass.AP(tensor=bo_d.tensor, offset=bo_d.offset,
                                ap=[[0, 128], [1, D]]))
                for nt in range(NT):
                    fp = psum.tile([128, D], F32, tag="bank")
                    for c in range(NC):
                        nc.tensor.matmul(
                            fp, ot[c][:, nt * 128:(nt + 1) * 128],
                            wo_bf[:, c, :],
                            start=(c == 0), stop=(c == NC - 1))
                    osb = outp.tile([128, D], F32, tag="osb")
                    nc.vector.tensor_add(osb, fp, bo128)
                    nc.sync.dma_start(
                        out=out_d[nt * 128:(nt + 1) * 128, :], in_=osb)
            otpool_cm.__exit__(None, None, None)

    nc.compile()
    return nc


_CACHED = None


def _get_nc():
    global _CACHED
    if _CACHED is None:
        _CACHED = build_kernel()
    return _CACHED


def run(inputs: dict, trace: bool = False):
    from concourse.bass_utils import run_bass_kernel_spmd
    nc = _get_nc()
    x = np.ascontiguousarray(inputs["x"], dtype=np.float32)
    b = x.shape[0]
    shared = {k: np.ascontiguousarray(inputs[k], dtype=np.float32)
              for k in ("Wq", "Wk", "Wv", "Wo", "bo")}
    in_maps = [{"x": x[i], **shared} for i in range(b)]
    res = run_bass_kernel_spmd(nc, in_maps, list(range(b)), trace=trace)
    out = np.stack([res.results[i]["out"] for i in range(b)], axis=0)
    return out, res


def kernel(**inputs) -> np.ndarray:
    out, _ = run(inputs, trace=False)
    return out.astype(np.float32)
